# revision 34
# baseline (speedup 1.0000x reference)
"""Trainium2 Bass kernel for nn_AttentionLayer (sparse_attention).

Math (per batch b, history l):
    info = [q, k, q-k, q*k] @ W1 + b1 ; @ W2 + b2 ; sigmoid ; @ Wf + bf
    score = softmax(where(mask, -inf, logit), axis=l)
    out   = sum_l score * v

Host-side algebra (exact up to fp assoc):
  - No nonlinearity between W1/W2  =>  h2 = k@P + (q*k)@Q + r_b
        P = (W1b-W1c)@W2, Q = W1d@W2, r_b = q_b@(W1a+W1c)@W2 + b1@W2 + b2
  - Fold q into per-batch weights: h2 = k @ V_b + r_b,  V_b = P + diag(q_b) Q
  - Fold r_b into k: solve s_b @ V_b = r_b (least-norm), ship k + s_b
  - sigmoid(x)@Wf = tanh(x/2)@(Wf/2) + const; const cancels in softmax
  - MASK COMPACTION: masked tokens (score exactly 0) are dropped on host;
    batches are sorted by unmasked count and binned into 4 quarters with
    slot counts LPS=[128,112,104,96] (pads: k=0 -> logit 0, madd=-30, v=0).
  - k-stream and folded weights ship in fp8e4m3 (softmax smooths the
    quantization); v ships bf16 (it hits the output linearly).
Device layout: token-major 2-stream columns (batch-pair -> LP cols,
partitions 0:64 = stream-A E-dims, 64:128 = stream-B). One block-diagonal
[128,80] fp8 matmul per pair -> h2 [80,LP] (A h2 parts 0:40, B 40:80); tanh
(scale .5) -> t bf16; wf matmuls [80,2] write logits into 4 PSUM partition
strips (32s, 32s+1); ACT-copy evacuates [98, 4LP] to bf16 staging; 4 DMAs
per quarter unfold logits batch-major [128, LP]; softmax + p@v on DVE
(exp w/ accum z on ACT; mult + 2 folds + reduce + scale).
"""

import sys

sys.path.insert(0, "/opt/trn_rl_repo")

import numpy as np
import ml_dtypes

import concourse.bass as bass
import concourse.bacc as bacc
import concourse.tile as tile
import concourse.mybir as mybir
from concourse.bass_utils import run_bass_kernel_spmd

N_CORES = 8
B_FULL = 4096
B = B_FULL // N_CORES   # 512 batches per core
E = 64
H = 40
LPS = [128, 112, 104, 96]   # history slots per quarter (sorted batches)
KB = [0]                    # kx col base per quarter
for _lp in LPS:
    KB.append(KB[-1] + 64 * _lp)
NKX = KB[4]                 # total kx columns (28160)
NPAIR = 256
GP = 8                      # pairs per group
# slabs: (quarter, first group in quarter, n groups)
SLABS = [(0, 0, 1), (0, 1, 1), (0, 2, 1), (0, 3, 1), (0, 4, 4),
         (1, 0, 4), (1, 4, 4), (2, 0, 4), (2, 4, 4), (3, 0, 4), (3, 4, 4)]

BF16 = mybir.dt.bfloat16
FP8 = mybir.dt.float8e4
I8 = mybir.dt.int8
F32 = mybir.dt.float32
nbf16 = ml_dtypes.bfloat16
nfp8 = ml_dtypes.float8_e4m3fn


def build_nc():
    nc = bacc.Bacc()

    kx_d = nc.declare_dram_parameter("kx", [128, NKX], FP8, isOutput=False)
    vw_d = nc.declare_dram_parameter("vw", [128, NPAIR * 80], FP8, isOutput=False)
    wf_d = nc.declare_dram_parameter("wf2", [80, 2], BF16, isOutput=False)
    vt_d = [nc.declare_dram_parameter(f"vt{q}", [128, E * LPS[q]], I8,
                                      isOutput=False) for q in range(4)]
    sr_d = [nc.declare_dram_parameter(f"sr{q}", [128, LPS[q]], BF16,
                                      isOutput=False) for q in range(4)]
    zo_d = [nc.declare_dram_parameter(f"zo{q}", [128, 1], F32,
                                      isOutput=False) for q in range(4)]
    out_d = nc.declare_dram_parameter("out", [B, E], F32, isOutput=True)

    Tanh = mybir.ActivationFunctionType.Tanh
    Exp = mybir.ActivationFunctionType.Exp
    Copy = mybir.ActivationFunctionType.Copy
    Alu = mybir.AluOpType
    X = mybir.AxisListType.X

    from contextlib import ExitStack

    with tile.TileContext(nc) as tc, ExitStack() as ctx:
        const = ctx.enter_context(tc.tile_pool(name="const", bufs=1))
        kxp = ctx.enter_context(tc.tile_pool(name="kxp", bufs=1))
        vwp = ctx.enter_context(tc.tile_pool(name="vwp", bufs=1))
        h2p = ctx.enter_context(tc.tile_pool(name="h2p", bufs=2, space="PSUM"))
        lgp = ctx.enter_context(tc.tile_pool(name="lgp", bufs=2, space="PSUM"))
        tp = ctx.enter_context(tc.tile_pool(name="tp", bufs=2))
        stp = ctx.enter_context(tc.tile_pool(name="stp", bufs=2))
        lmp = ctx.enter_context(tc.tile_pool(name="lmp", bufs=2))
        vtp = ctx.enter_context(tc.tile_pool(name="vtp", bufs=3))
        srp = ctx.enter_context(tc.tile_pool(name="srp", bufs=2))
        bp = ctx.enter_context(tc.tile_pool(name="bp", bufs=1))

        wf_t = const.tile([80, 2], BF16, tag="wf")
        nc.sync.dma_start(wf_t[:], wf_d[:])

        kx_t = {}
        vw_t = {}

        # global group -> slab index; global first group of each slab
        g2slab = {}
        slab_g0 = []
        for si, (sq, g0q, ng) in enumerate(SLABS):
            slab_g0.append(8 * sq + g0q)
            for j in range(ng):
                g2slab[8 * sq + g0q + j] = si

        def load_slab(si):
            sq, g0q, ng = SLABS[si]
            lp = LPS[sq]
            c0 = KB[sq] + g0q * GP * lp
            ncol = ng * GP * lp
            kt = kxp.tile([128, ncol], FP8, tag=f"kx{si}", name=f"kx{si}")
            nc.sync.dma_start(kt[:], kx_d[:, c0:c0 + ncol])
            kx_t[si] = kt
            r0 = 64 * sq + g0q * GP
            wt = vwp.tile([128, ng * GP * 80], FP8, tag=f"vw{si}", name=f"vw{si}")
            nc.sync.dma_start(wt[:], vw_d[:, r0 * 80:(r0 + ng * GP) * 80])
            vw_t[si] = wt

        qdat = {}
        pend = {}

        def load_quarter(qq, chunk):
            lp = LPS[qq]
            if chunk == 0:
                vt_t = vtp.tile([128, E * 128], BF16, tag="vt", name=f"vt{qq}")
                sr_t = srp.tile([128, 128], BF16, tag="sr", name=f"sr{qq}")
                zo_t = srp.tile([128, 1], F32, tag="zo", name=f"zo{qq}")
                nc.gpsimd.dma_start(sr_t[:, 0:lp], sr_d[qq][:])
                nc.gpsimd.dma_start(zo_t[:], zo_d[qq][:])
                qdat[qq] = (vt_t, sr_t, zo_t)
            vt_t = qdat[qq][0]
            qc = E * lp // 8
            for h in range(2):
                c0 = (2 * chunk + h) * qc
                # int8 -> bf16 cast rides the SWDGE DMA for free
                nc.gpsimd.dma_start(vt_t[:, c0:c0 + qc], vt_d[qq][:, c0:c0 + qc])

        def phase_b_pre(qq, lm_t):
            pend[qq] = (qdat.pop(qq), lm_t)

        def phase_b(qq):
            (vt_t, sr_t, zo_t), lm_t = pend.pop(qq)
            lp = LPS[qq]
            p_t = bp.tile([128, 128], BF16, tag="p", name=f"p{qq}")
            nc.scalar.activation(p_t[:, 0:lp], lm_t[:, 0:lp], Exp)

            # z = sum(p) - n_pad  (pads give exp(0)=1 exactly)
            zr = bp.tile([128, 1], F32, tag="zr", name=f"zr{qq}")
            nc.vector.tensor_reduce(
                zr[:], p_t[:, 0:lp].rearrange("p (o l) -> p o l", o=1),
                axis=X, op=Alu.add)
            # p' = p * srow (srow = smax/127, 0 on pads)
            pp_t = bp.tile([128, 128], BF16, tag="pp", name=f"pp{qq}")
            nc.vector.tensor_tensor(
                pp_t[:, 0:lp], p_t[:, 0:lp], sr_t[:, 0:lp], Alu.mult)

            w1 = bp.tile([128, E * 128], BF16, tag="w1", name=f"w1{qq}")
            p_b = pp_t[:, 0:lp].rearrange("p (o l) -> p o l", o=1) \
                               .broadcast_to([128, E, lp])
            nc.vector.tensor_tensor(
                w1[:, 0:E * lp].rearrange("p (e l) -> p e l", e=E),
                vt_t[:, 0:E * lp].rearrange("p (e l) -> p e l", e=E),
                p_b, Alu.mult,
            )
            w2 = bp.tile([128, E * 64], BF16, tag="w2", name=f"w2{qq}")
            w1v = w1[:, 0:E * lp].rearrange("p (e l) -> p e l", e=E)
            nc.vector.tensor_tensor(
                w2[:, 0:E * lp // 2].rearrange("p (e l) -> p e l", e=E),
                w1v[:, :, 0:lp // 2], w1v[:, :, lp // 2:lp], Alu.add,
            )
            w3 = bp.tile([128, E * 32], BF16, tag="w3", name=f"w3{qq}")
            w2v = w2[:, 0:E * lp // 2].rearrange("p (e l) -> p e l", e=E)
            nc.vector.tensor_tensor(
                w3[:, 0:E * lp // 4].rearrange("p (e l) -> p e l", e=E),
                w2v[:, :, 0:lp // 4], w2v[:, :, lp // 4:lp // 2], Alu.add,
            )
            acc = bp.tile([128, E], F32, tag="acc", name=f"acc{qq}")
            nc.vector.tensor_reduce(
                acc[:], w3[:, 0:E * lp // 4].rearrange("p (e l) -> p e l", e=E),
                axis=X, op=Alu.add)
            zc = bp.tile([128, 1], F32, tag="zc", name=f"zc{qq}")
            nc.vector.tensor_tensor(zc[:], zr[:], zo_t[:], Alu.subtract)
            rz = bp.tile([128, 1], F32, tag="rz", name=f"rz{qq}")
            nc.vector.reciprocal(rz[:], zc[:])
            o_t = bp.tile([128, E], F32, tag="o", name=f"o{qq}")
            nc.vector.tensor_scalar_mul(o_t[:], acc[:], rz[:])
            nc.gpsimd.dma_start(out_d[qq * 128:(qq + 1) * 128, :], o_t[:])

        load_slab(0)
        st_t = None
        lg_t = None
        for g in range(32):
            if g % 8 == 2 and g // 8 - 1 in pend:
                phase_b(g // 8 - 1)
            si = g2slab[g]
            if g == slab_g0[si] and si + 1 < len(SLABS):
                load_slab(si + 1)
            if g == 2:
                # gate the vt stream behind slab-1 arrival: without this the
                # idle gpsimd ring dispatches vt0 (2.4MB) at t~7us and starves
                # the first kx slabs of HBM bandwidth
                gate = bp.tile([1, 8], FP8, tag="gate", name="gate")
                nc.gpsimd.tensor_copy(gate[:], kx_t[1][0:1, 0:8])
            if 4 <= g < 8:
                load_quarter(0, g - 4)
            elif g >= 8 and 2 <= g % 8 < 6:
                load_quarter(g // 8, g % 8 - 2)

            qq = g // 8
            lp = LPS[qq]
            gc = GP * lp                      # group columns
            kxs, vws = kx_t[si], vw_t[si]
            sq, g0q, _ = SLABS[si]
            h2_t = h2p.tile([80, GP * 128], F32, tag="h2", name=f"h2_{g}")
            for pp in range(GP):
                rr = (g - 8 * sq - g0q) * GP + pp   # pair within slab
                nc.tensor.matmul(
                    h2_t[0:80, pp * lp:(pp + 1) * lp],
                    vws[:, rr * 80:rr * 80 + 80],
                    kxs[:, rr * lp:(rr + 1) * lp],
                    start=True, stop=True,
                )
            t_t = tp.tile([80, GP * 128], BF16, tag="t", name=f"t_{g}")
            nc.scalar.activation(
                t_t[:, 0:gc], h2_t[0:80, 0:gc], Tanh, scale=0.5)

            if g % 2 == 0:
                lg_t = lgp.tile([98, 512], F32, tag="lg", name=f"lg_{g // 2}")
            for j in range(2):
                ss = 2 * (g % 2) + j
                nc.tensor.matmul(
                    lg_t[32 * ss:32 * ss + 2, 0:4 * lp],
                    wf_t[:], t_t[:, j * 4 * lp:(j + 1) * 4 * lp],
                    start=True, stop=True, tile_position=(0, 32 * ss),
                )
            if g % 2 == 1:
                gq = (g // 2) % 4
                if gq == 0:
                    st_t = stp.tile([98, 4 * 512], BF16, tag="st", name=f"st{qq}")
                if gq == 1:
                    nc.vector.tensor_copy(
                        st_t[:, 4 * lp * gq:4 * lp * (gq + 1)],
                        lg_t[:, 0:4 * lp])
                else:
                    nc.scalar.activation(
                        st_t[:, 4 * lp * gq:4 * lp * (gq + 1)],
                        lg_t[:, 0:4 * lp], Copy)

                if gq == 3:
                    lm_t = lmp.tile([128, 128], BF16, tag="lm", name=f"lm{qq}")
                    for ss in range(4):
                        deng = nc.scalar if (qq == 3 and ss % 2 == 0) \
                            else nc.gpsimd
                        deng.dma_start(
                            lm_t[32 * ss:32 * ss + 32, 0:lp],
                            st_t[32 * ss:32 * ss + 2, 0:16 * lp])
                    phase_b_pre(qq, lm_t)
        phase_b(3)

    if not nc.is_finalized():
        nc.finalize()
    return nc


def host_prep(q, k, v, mask, W1, b1, W2, b2, Wf, bf):
    """Fold weights per batch, compact masked tokens, build device layouts."""
    q2 = q[:, 0, :].astype(np.float32)
    W1 = W1.astype(np.float32); W2 = W2.astype(np.float32)
    P = (W1[64:128] - W1[128:192]) @ W2
    Q = W1[192:256] @ W2
    A2 = (W1[0:64] + W1[128:192]) @ W2
    c0 = b1.astype(np.float32) @ W2 + b2.astype(np.float32)
    r = q2 @ A2 + c0
    V = P[None] + q2[:, :, None] * Q[None]                  # [Bf,64,40]
    G = np.einsum('beh,bei->bhi', V, V)
    y = np.linalg.solve(G, r[:, :, None])
    s = np.einsum('beh,bhx->be', V, y)

    m = mask[:, :, 0]
    order = np.argsort(m, axis=1, kind='stable')[:, :128]
    nvalid = (~m).sum(1)
    validc = np.arange(128)[None, :] < nvalid[:, None]
    kc = np.take_along_axis(k.astype(np.float32), order[:, :, None], 1)
    vc = np.take_along_axis(v.astype(np.float32), order[:, :, None], 1)
    kc = np.where(validc[..., None], kc + s[:, None, :], 0.0)
    vc = np.where(validc[..., None], vc, 0.0)
    # per-slot int8 scale for v; srow folds into p, zoff corrects z for pads
    smax = np.maximum(np.abs(vc).max(axis=2), np.float32(1e-6))
    v8 = np.round(vc / smax[:, :, None] * 127).clip(-127, 127).astype(np.int8)
    srow = np.where(validc, smax / 127, 0.0).astype(nbf16)

    # pair rq -> within-quarter positions of its A/B batches
    rqi = np.arange(64)
    tA = 32 * ((rqi % 16) // 4) + 4 * (rqi // 16) + rqi % 4
    tB = tA + 16

    in_maps = []
    invs = []
    for c in range(N_CORES):
        sl = slice(c * B, (c + 1) * B)
        ncore = nvalid[sl]
        ordr = np.argsort(-ncore, kind='stable')   # desc by unmasked count
        invs.append(ordr)
        kcc, vcc, Vc = kc[sl], v8[sl], V[sl]
        kx = np.zeros((128, NKX), np.float32)
        vw3 = np.zeros((NPAIR, 128, 80), np.float32)
        imap = {}
        for qq in range(4):
            lp = LPS[qq]
            ranks = ordr[128 * qq:128 * (qq + 1)]
            assert ncore[ranks].max() <= lp, \
                f"core {c} q{qq}: {ncore[ranks].max()} > {lp}"
            Ab = ranks[tA]
            Bb = ranks[tB]
            kx[0:64, KB[qq]:KB[qq + 1]] = \
                kcc[Ab][:, :lp].transpose(2, 0, 1).reshape(64, -1)
            kx[64:128, KB[qq]:KB[qq + 1]] = \
                kcc[Bb][:, :lp].transpose(2, 0, 1).reshape(64, -1)
            vw3[64 * qq:64 * (qq + 1), 0:64, 0:40] = Vc[Ab]
            vw3[64 * qq:64 * (qq + 1), 64:128, 40:80] = Vc[Bb]
            imap[f"vt{qq}"] = np.ascontiguousarray(
                vcc[ranks][:, :lp].transpose(0, 2, 1)).reshape(128, E * lp)
            imap[f"sr{qq}"] = np.ascontiguousarray(srow[sl][ranks][:, :lp])
            imap[f"zo{qq}"] = (lp - ncore[ranks].astype(np.float32)
                               )[:, None].astype(np.float32)
        vw = vw3.transpose(1, 0, 2).reshape(128, NPAIR * 80)
        wf2 = np.zeros((80, 2), np.float32)
        wf2[0:40, 0] = 0.5 * Wf[:, 0]
        wf2[40:80, 1] = 0.5 * Wf[:, 0]
        imap["kx"] = np.ascontiguousarray(kx).astype(nfp8)
        imap["vw"] = np.ascontiguousarray(vw).astype(nfp8)
        imap["wf2"] = wf2.astype(nbf16)
        in_maps.append(imap)
    return in_maps, invs


_CACHE = {}


def run_on_device(in_maps, trace=False):
    if "nc" not in _CACHE:
        _CACHE["nc"] = build_nc()
    nc = _CACHE["nc"]
    res = run_bass_kernel_spmd(nc, in_maps, core_ids=list(range(N_CORES)),
                               trace=trace)
    return res


def gather_out(res, invs):
    outs = []
    for c in range(N_CORES):
        oc = np.empty((B, E), np.float32)
        oc[invs[c]] = res.results[c]["out"]
        outs.append(oc)
    return np.concatenate(outs, axis=0)


def kernel(q, k, v, mask, W1, b1, W2, b2, Wf, bf):
    in_maps, invs = host_prep(q, k, v, mask, W1, b1, W2, b2, Wf, bf)
    res = run_on_device(in_maps)
    return gather_out(res, invs).astype(np.float32)
